# revision 31
# baseline (speedup 1.0000x reference)
"""Trainium2 Bass kernel for BatchPrototypeHead (segment_reduce).

Math (reference):
    q = relu(query @ W1.T + b1) @ W2.T + b2          (B, d)
    s = relu(support @ W1.T + b1) @ W2.T + b2        (S, d)
    protos[c] = mean of s rows with label c (0 if empty)
    out[b, c] = -||q_b - protos_c||^2

Kernel strategy (8 NeuronCores, SPMD):
  - Shard both query (B) and support (S) over the 8 cores.
  - Inputs are host-packed so each tensor loads with ONE dma_start (the
    HWDGE sync queue issues serially at ~0.6us per call, and the
    runtime's collective entry barrier is queued behind them).
  - Support side per core: h = relu(x @ W1.T) (rows on partitions),
    one-hot(labels) built on-device (iota + is_equal).  Segment sums use
    the one-hot as the STATIONARY matmul operand: one accumulating matmul
    per 128-row chunk, out[100, 257] += onehot.T @ [h | ones]  (the ones
    column yields per-class counts in col 256).  W2 is applied once to
    the reduced (100, 256) sums (associativity).
  - AllGather the (100, 260) f16 partial [sums | counts] across cores,
    8-way summed locally with one DVE strided reduce.
  - b2 cancels exactly in ||q - p||^2; b1 applied via the free ACT bias
    on the query side and (only if b1 != 0, trace-time branch) an extra
    accumulating matmul on the support side.
  - Post-AG tail: scale sums by 2/max(cnt,1) (per-partition scalar since
    counts land as a [100,1] column), PE-transpose to [d, c], apply W2,
    then distances with the PROTOTYPES as stationary operands and the
    query streaming 512 columns at a time (6 matmuls total):
        G[c, b] = 2 p.q - pn - qn   accumulated per 512-query block
    The output is produced transposed ([C, BL] per core) and un-transposed
    on the host after the gather.
  - dtypes: support x / W1 / h / onehot in bf16; query chain + prototype
    path in float32r.  DRAM inputs on the f32r path are pre-rounded on
    the host (BIR requires f32r matmul operands produced as f32r).
"""

import numpy as np
import ml_dtypes
from contextlib import ExitStack

import concourse.bass as bass
import concourse.bacc as bacc
import concourse.mybir as mybir
import concourse.tile as tile
from concourse import bass_utils, masks

BF16 = mybir.dt.bfloat16
F32 = mybir.dt.float32
F32R = mybir.dt.float32r
I32 = mybir.dt.int32
F16 = mybir.dt.float16
F8 = mybir.dt.float8e4
AF = mybir.ActivationFunctionType
OP = mybir.AluOpType
AX = mybir.AxisListType

B, S, D, C = 8192, 65536, 256, 100
NCORES = 8
BL, SL = B // NCORES, S // NCORES          # 1024 query rows, 8192 support rows
NCH = SL // 128                            # 64 support chunks / core
CP = 260                                   # padded AG cols
CR = 104                                   # padded AG rows (104*260 % 32 == 0)


def f32r_round(x):
    """Round fp32 values to the fp32r grid (nearest, ~12-bit mantissa)."""
    x = np.ascontiguousarray(x, dtype=np.float32)
    try:
        from neuron_dtypes._impl import fp32r as _m
        u = x.reshape(-1).view(np.uint32)
        r = np.asarray(_m.cast_fp32_to_fp32r(len(u), u), dtype=np.uint32)
        b = np.asarray(_m.cast_fp32r_to_fp32(len(r), r), dtype=np.uint32)
        return b.view(np.float32).reshape(x.shape)
    except Exception:
        u = x.reshape(-1).view(np.uint32).astype(np.uint64)
        r = ((u + 0x800) & ~np.uint64(0xFFF)).astype(np.uint32)
        return r.view(np.float32).reshape(x.shape)


def pack_halves(a):
    """[256, N] -> [128, 2*N]: row p holds [k=0 half | k=1 half]."""
    n = a.shape[1]
    return np.ascontiguousarray(
        a.reshape(2, 128, n).transpose(1, 0, 2).reshape(128, 2 * n))


def build_program(b1_nonzero: bool):
    nc = bacc.Bacc("TRN2", target_bir_lowering=False, debug=False,
                   num_devices=NCORES)

    # packed inputs: one dma_start per tensor
    xt = nc.dram_tensor("xt_sup", [128, 2 * SL], F8,
                        kind="ExternalInput").ap()   # chunk-major [ci, k, j]
    lab = nc.dram_tensor("lab", [128, NCH], F32, kind="ExternalInput").ap()
    xqt = nc.dram_tensor("xqt", [128, 2 * BL], F32R,
                         kind="ExternalInput").ap()
    w1tb = nc.dram_tensor("w1t_bf", [128, 2 * D], F8,
                          kind="ExternalInput").ap()
    w1tf = nc.dram_tensor("w1t_f", [128, 2 * D], F32R,
                          kind="ExternalInput").ap()
    w2tf = nc.dram_tensor("w2t_f", [128, 2 * D], F32R,
                          kind="ExternalInput").ap()
    b1c = nc.dram_tensor("b1c", [128, 2], F32, kind="ExternalInput").ap()
    b1rb = nc.dram_tensor("b1r_bf", [1, D], BF16, kind="ExternalInput").ap()
    out = nc.dram_tensor("out", [C, BL], F16, kind="ExternalOutput").ap()

    with tile.TileContext(nc) as tc, ExitStack() as ctx:
        sb = ctx.enter_context(tc.tile_pool(name="sb", bufs=1))
        dram = ctx.enter_context(tc.tile_pool(name="dram", bufs=1, space="DRAM"))
        hpool = ctx.enter_context(tc.tile_pool(name="hpool", bufs=3))
        ohpool = ctx.enter_context(tc.tile_pool(name="ohpool", bufs=3))
        opool = ctx.enter_context(tc.tile_pool(name="opool", bufs=2))

        # ---- weights / support stream (9 serial dma_start issuances total)
        w1tb_sb = sb.tile([128, 2 * D], F8, name="w1tb_sb")
        nc.sync.dma_start(w1tb_sb[:], w1tb[:])
        lab_sb = sb.tile([128, NCH], F32, name="lab_sb")
        nc.sync.dma_start(lab_sb[:], lab[:])
        xt_sb = sb.tile([128, 2 * SL], F8, name="xt_sb")
        nc.sync.dma_start(xt_sb[:, 0:2048], xt[:, 0:2048])
        nc.sync.dma_start(xt_sb[:, 2048:8192], xt[:, 2048:8192])
        nc.sync.dma_start(xt_sb[:, 8192:2 * SL], xt[:, 8192:2 * SL])
        xqt_sb = sb.tile([128, 2 * BL], F32R, name="xqt_sb")
        nc.sync.dma_start(xqt_sb[:], xqt[:])
        w1tf_sb = sb.tile([128, 2 * D], F32R, name="w1tf_sb")
        nc.sync.dma_start(w1tf_sb[:], w1tf[:])
        w2tf_sb = sb.tile([128, 2 * D], F32R, name="w2tf_sb")
        nc.sync.dma_start(w2tf_sb[:], w2tf[:])
        b1c_sb = sb.tile([128, 2], F32, name="b1c_sb")
        nc.sync.dma_start(b1c_sb[:], b1c[:])

        iota_i = sb.tile([128, C], I32, name="iota_i")
        nc.gpsimd.iota(iota_i[:], pattern=[[1, C]], base=0, channel_multiplier=0)
        iota_f = sb.tile([128, C], F32, name="iota_f")
        nc.vector.tensor_copy(iota_f[:], iota_i[:])
        if b1_nonzero:
            ones_row_bf = sb.tile([1, 128], BF16, name="ones_row_bf")
            nc.vector.memset(ones_row_bf[:], 1.0)
            b1rb_sb = sb.tile([1, D], BF16, name="b1rb_sb")
            nc.sync.dma_start(b1rb_sb[:], b1rb[:])

        # ---- PE warm-up: dummy matmuls on a memset tile so the HAM clock
        # gate flips to 2.4 GHz before the real work arrives
        scratch_bf = sb.tile([128, 512], BF16, name="scratch_bf")
        nc.vector.memset(scratch_bf[:], 0.5)
        with tc.tile_pool(name="pwarm", bufs=1, space="PSUM") as pwarm:
            warm_ps = pwarm.tile([128, 512], F32, name="warm_ps")
            for _ in range(5):
                nc.tensor.matmul(warm_ps[:], scratch_bf[:, 0:128],
                                 scratch_bf[:], start=True, stop=True,
                                 skip_group_check=True)

        def x_slice(k, ci):
            base = ci * 256 + k * 128
            return xt_sb[:, base:base + 128]

        # ---- distance-stage f32r constants (DVE copy from f32 memsets,
        # which the BIR verifier accepts as a valid f32r producer)
        onesq_f = sb.tile([1, BL], F32, name="onesq_f")
        nc.vector.memset(onesq_f[:], 1.0)
        ones_q = sb.tile([1, BL], F32R, name="ones_q")
        nc.vector.tensor_copy(ones_q[:], onesq_f[:])
        e1f = sb.tile([1, 2], F32, name="e1f")
        nc.vector.memset(e1f[:, 0:1], 0.0)
        nc.vector.memset(e1f[:, 1:2], 1.0)
        e1_r = sb.tile([1, 2], F32R, name="e1_r")
        nc.vector.tensor_copy(e1_r[:], e1f[:])
        cm025_2 = sb.tile([128, 2], F32, name="cm025_2")
        nc.vector.memset(cm025_2[:, 0:1], 0.0)
        nc.vector.memset(cm025_2[:, 1:2], -0.25)
        em1 = sb.tile([1, 2], F32, name="em1")
        nc.vector.memset(em1[:, 0:1], -1.0)
        nc.vector.memset(em1[:, 1:2], 0.0)
        ones_c = sb.tile([1, C], F32, name="ones_c")
        nc.vector.memset(ones_c[:], 1.0)
        oc2_f = sb.tile([128, 2], F32, name="oc2_f")
        nc.vector.memset(oc2_f[:, 0:1], 1.0)
        nc.vector.memset(oc2_f[:, 1:2], 0.0)
        onescol2 = sb.tile([128, 2], F32R, name="onescol2")
        nc.vector.tensor_copy(onescol2[:], oc2_f[:])
        ident = sb.tile([128, 128], F32, name="ident")
        masks.make_identity(nc, ident[:])

        # partial [sums | cnt | pad] staged for the AllGather, on an fp8
        # wire: values scaled by 1/64 into e4m3 range (sums are 16x from
        # the W1 host scaling; class sums stay well under 240*64)
        part_sb = sb.tile([CR, CP], F8, name="part_sb")
        nc.vector.memset(part_sb[:], 0.0)

        # ================= support phase =================
        # seg_ps[100, 257] accumulates onehot.T @ [h | ones] over all 64
        # chunks: cols 0:256 are per-class h sums, col 256 the counts.
        with tc.tile_pool(name="ph", bufs=3, space="PSUM") as ph, \
             tc.tile_pool(name="pacc", bufs=1, space="PSUM") as pacc:
            seg_ps = pacc.tile([C, D + 1], F32, name="seg_ps")
            for ci in range(NCH):
                h_ps = ph.tile([128, D], F32, name="h_ps")
                nc.tensor.matmul(h_ps[:], x_slice(0, ci), w1tb_sb[:, 0:D],
                                 start=True, stop=False)
                nc.tensor.matmul(h_ps[:], x_slice(1, ci), w1tb_sb[:, D:2 * D],
                                 start=False, stop=not b1_nonzero)
                if b1_nonzero:
                    nc.tensor.matmul(h_ps[:], ones_row_bf[:], b1rb_sb[:],
                                     start=False, stop=True)
                h_sb = hpool.tile([128, D + 1], BF16, name="h_sb")
                nc.scalar.activation(h_sb[:, 0:D], h_ps[:], AF.Relu)
                nc.vector.memset(h_sb[:, D:D + 1], 1.0)
                oh = ohpool.tile([128, C], BF16, name="oh")
                nc.vector.tensor_scalar(oh[:], iota_f[:], lab_sb[:, ci:ci + 1],
                                        None, OP.is_equal)
                nc.tensor.matmul(seg_ps[:], oh[:], h_sb[:],
                                 start=ci == 0, stop=ci == NCH - 1)
            nc.vector.tensor_scalar(part_sb[0:C, 0:D + 1], seg_ps[:],
                                    1.0 / 64.0, None, OP.mult)

        # ========== AllGather partial [sums | cnt], reduce locally ======
        arin = dram.tile([CR, CP], F8, name="arin")
        arout = dram.tile([NCORES * CR, CP], F8, name="arout")
        nc.sync.dma_start(arin[:], part_sb[:])
        nc.gpsimd.collective_compute(
            "AllGather", OP.bypass,
            replica_groups=[list(range(NCORES))],
            ins=[arin.opt()], outs=[arout.opt()])

        # ================= query projector (overlaps the AG window) ======
        hq_sb = [sb.tile([128, BL], F32R, name=f"hq{k}") for k in range(2)]
        qt_sb = [sb.tile([128, BL], F32R, name=f"qt{k}") for k in range(2)]
        qsq_sb = [sb.tile([128, BL], F32R, name=f"qsq{k}") for k in range(2)]
        # qn2: row 0 = ||q||^2, row 1 = ones -> distance rhs K-chunk of 2
        qn2 = sb.tile([2, BL], F32R, name="qn2")
        with tc.tile_pool(name="pq", bufs=2, space="PSUM") as pq:
            for qb in range(2):
                qsl = slice(qb * 512, (qb + 1) * 512)
                for mc in range(2):
                    msl = slice(mc * 128, (mc + 1) * 128)
                    hq_ps = pq.tile([128, 512], F32, name="hq_ps")
                    for kc in range(2):
                        nc.tensor.matmul(
                            hq_ps[:],
                            w1tf_sb[:, kc * D + mc * 128:kc * D + mc * 128 + 128],
                            xqt_sb[:, kc * BL + qb * 512:kc * BL + qb * 512 + 512],
                            start=kc == 0, stop=kc == 1)
                    nc.scalar.activation(hq_sb[mc][:, qsl], hq_ps[:], AF.Relu,
                                         bias=b1c_sb[:, mc:mc + 1])
                for mc in range(2):
                    qt_ps = pq.tile([128, 512], F32, name="qt_ps")
                    for kc in range(2):
                        nc.tensor.matmul(
                            qt_ps[:],
                            w2tf_sb[:, kc * D + mc * 128:kc * D + mc * 128 + 128],
                            hq_sb[kc][:, qsl],
                            start=kc == 0, stop=kc == 1)
                    nc.scalar.copy(qt_sb[mc][:, qsl], qt_ps[:])
                    nc.scalar.square(qsq_sb[mc][:, qsl], qt_ps[:])
                qn_ps = pq.tile([2, 512], F32, name="qn_ps")
                for kc in range(2):
                    nc.tensor.matmul(qn_ps[:], onescol2[:],
                                     qsq_sb[kc][:, qsl],
                                     start=kc == 0, stop=False)
                nc.tensor.matmul(qn_ps[:], e1_r[:], ones_q[:, qsl],
                                 start=False, stop=True)
                nc.scalar.copy(qn2[:, qsl], qn_ps[:])

        # ================= combine partials (needs the AG result) =========
        ar_view = arout.rearrange("(r z) j -> r z j", z=CR)
        gk = sb.tile([C, NCORES, CP], F8, name="gk")
        nc.sync.dma_start(gk[:],
                          ar_view[:, 0:C, :].rearrange("r p j -> p r j"))
        # count column reduced first in a tiny parallel chain so the
        # reciprocal is ready before the big add tree finishes
        c1 = sb.tile([C, 4, 1], F32, name="c1")
        nc.vector.tensor_tensor(c1[:], gk[:, 0:4, D:D + 1],
                                gk[:, 4:8, D:D + 1], OP.add)
        c2t = sb.tile([C, 2, 1], F32, name="c2t")
        nc.vector.tensor_tensor(c2t[:], c1[:, 0:2, :], c1[:, 2:4, :], OP.add)
        ccol = sb.tile([C, 1], F32, name="ccol")
        nc.vector.tensor_tensor(ccol[:], c2t[:, 0:1, :].squeeze(1),
                                c2t[:, 1:2, :].squeeze(1), OP.add)
        a1 = sb.tile([C, 4, CP], F16, name="a1")
        nc.vector.tensor_tensor(a1[:], gk[:, 0:4, :], gk[:, 4:8, :], OP.add)
        a2 = sb.tile([C, 2, CP], F16, name="a2")
        nc.vector.tensor_tensor(a2[:], a1[:, 0:2, :], a1[:, 2:4, :], OP.add)
        hsr = sb.tile([C, CP], F32, name="hsr")
        nc.vector.tensor_tensor(hsr[:], a2[:, 0:1, :].squeeze(1),
                                a2[:, 1:2, :].squeeze(1), OP.add)

        # ---- prototypes: scale by 2/max(cnt,1), transpose, apply W2
        cmax = sb.tile([C, 1], F32, name="cmax")
        inv2 = sb.tile([C, 1], F32, name="inv2")
        hmr = [sb.tile([128, C], F32R, name=f"hmr{k}") for k in range(2)]
        p2t_r = [sb.tile([128, C], F32R, name=f"p2tr{k}") for k in range(2)]
        p2sq_sb = [sb.tile([128, C], F32, name=f"p2sq{k}") for k in range(2)]
        # pnm: row 0 = -ones, row 1 = -||p||^2 -> distance lhsT K-chunk of 2
        pnm = sb.tile([2, C], F32R, name="pnm")
        # distances are interleaved with the pnm preparation: the 4 big
        # query-x-prototype matmuls only need p2t_r, so the PE runs them
        # while ACT squares the prototypes and the small pnm matmuls
        # follow; each distance PSUM group closes with its pnm term.
        with tc.tile_pool(name="pp", bufs=1, space="PSUM") as pp:
            # cmax = max(cnt, 1) * 8 (fp8 wire + W1x16 scales folded in);
            # the per-class scale rides the transpose matmul as diag(inv2)
            nc.vector.tensor_scalar(cmax[:], ccol[:], 8.0, 0.125,
                                    OP.mult, OP.max)
            nc.vector.reciprocal(inv2[:], cmax[:])
            diag_sb = sb.tile([C, C], F32, name="diag_sb")
            nc.vector.tensor_scalar(diag_sb[:], ident[:C, :C], inv2[:],
                                    None, OP.mult)
            for kc in range(2):
                ht_ps = pp.tile([128, C], F32, name=f"ht{kc}")
                # plain fp32 matmul: hsr_slice.T @ diag(inv2) transposes AND
                # applies the per-class scale in one PE op
                nc.tensor.matmul(ht_ps[:], hsr[:, kc * 128:(kc + 1) * 128],
                                 diag_sb[:], start=True, stop=True)
                nc.vector.tensor_copy(hmr[kc][:], ht_ps[:])
            for mc in range(2):
                p2_ps = pp.tile([128, C], F32, name=f"p2_{mc}")
                for kc in range(2):
                    nc.tensor.matmul(
                        p2_ps[:],
                        w2tf_sb[:, kc * D + mc * 128:kc * D + mc * 128 + 128],
                        hmr[kc][:],
                        start=kc == 0, stop=kc == 1)
                nc.vector.tensor_copy(p2t_r[mc][:], p2_ps[:])
                nc.scalar.square(p2sq_sb[mc][:], p2_ps[:])
            g_ps = [pp.tile([C, 512], F32, name=f"g{h}") for h in range(2)]
            for half in range(2):
                qsl = slice(half * 512, (half + 1) * 512)
                nc.tensor.matmul(g_ps[half][:], p2t_r[0][:], qt_sb[0][:, qsl],
                                 start=True, stop=False)
                nc.tensor.matmul(g_ps[half][:], p2t_r[1][:], qt_sb[1][:, qsl],
                                 start=False, stop=False)
            mpn_ps = pp.tile([2, C], F32, name="mpn_ps")
            for mc in range(2):
                nc.tensor.matmul(mpn_ps[:], cm025_2[:], p2sq_sb[mc][:],
                                 start=mc == 0, stop=False)
            nc.tensor.matmul(mpn_ps[:], em1[:], ones_c[:],
                             start=False, stop=True)
            nc.vector.tensor_copy(pnm[:], mpn_ps[:])
            for half in range(2):
                qsl = slice(half * 512, (half + 1) * 512)
                nc.tensor.matmul(g_ps[half][:], pnm[:], qn2[:, qsl],
                                 start=False, stop=True)
                o_sb = opool.tile([C, 512], F16, name="o_sb")
                nc.vector.tensor_copy(o_sb[:], g_ps[half][:])
                nc.sync.dma_start(out[:, qsl], o_sb[:])

    nc.compile()
    return nc


def make_in_maps(query, support_feats, support_labels, W1, b1, W2, b2):
    q = np.ascontiguousarray(np.asarray(query, dtype=np.float32))
    x = np.asarray(support_feats, dtype=np.float32)
    labels = np.asarray(support_labels).astype(np.int64)
    W1 = np.asarray(W1, dtype=np.float32)
    b1 = np.asarray(b1, dtype=np.float32)
    W2 = np.asarray(W2, dtype=np.float32)

    w1t_f8 = pack_halves(np.ascontiguousarray(W1.T * 16.0)).astype(
        ml_dtypes.float8_e4m3)
    w1t_f = f32r_round(pack_halves(np.ascontiguousarray(W1.T)))
    w2t_f = f32r_round(pack_halves(np.ascontiguousarray(W2.T)))
    b1col = pack_halves(np.ascontiguousarray(b1.reshape(D, 1)))
    b1row_bf = np.ascontiguousarray(16.0 * b1.reshape(1, D)).astype(
        ml_dtypes.bfloat16)

    in_maps = []
    for c in range(NCORES):
        xs = x[c * SL:(c + 1) * SL]
        ls = labels[c * SL:(c + 1) * SL]
        xqs = q[c * BL:(c + 1) * BL]
        # xt chunk-major: [p, ci*256 + k*128 + j] = xs[ci*128 + j, k*128 + p]
        xt_pack = np.ascontiguousarray(
            xs.T.reshape(2, 128, NCH, 128).transpose(1, 2, 0, 3)
            .reshape(128, 2 * SL)).astype(ml_dtypes.float8_e4m3)
        in_maps.append({
            "xt_sup": xt_pack,
            "lab": np.ascontiguousarray(
                ls.reshape(NCH, 128).T.astype(np.float32)),
            "xqt": f32r_round(pack_halves(np.ascontiguousarray(xqs.T))),
            "w1t_bf": w1t_f8,
            "w1t_f": w1t_f,
            "w2t_f": w2t_f,
            "b1c": b1col,
            "b1r_bf": b1row_bf,
        })
    return in_maps


_cached = {}


def _get_program(b1_nonzero: bool):
    key = bool(b1_nonzero)
    if key not in _cached:
        _cached[key] = build_program(key)
    return _cached[key]


def kernel(query, support_feats, support_labels, W1, b1, W2, b2,
           **run_kwargs):
    b1_nonzero = bool(np.any(np.asarray(b1)))
    nc = _get_program(b1_nonzero)
    in_maps = make_in_maps(query, support_feats, support_labels,
                           W1, b1, W2, b2)
    res = bass_utils.run_bass_kernel_spmd(
        nc, in_maps, core_ids=list(range(NCORES)), **run_kwargs)
    out = np.concatenate(
        [np.ascontiguousarray(res.results[c]["out"].T) for c in range(NCORES)],
        axis=0)
    return out.astype(np.float32, copy=False)


if __name__ == "__main__":
    import sys
    sys.path.insert(0, "/root/problem")
    from reference import setup_inputs
    inputs = {k: np.asarray(v) for k, v in setup_inputs().items()}
    o = kernel(**inputs)
    print("out", o.shape, o.dtype, o[:2, :4])


# revision 32
# speedup vs baseline: 1.4617x; 1.4617x over previous
"""Trainium2 Bass kernel for BatchPrototypeHead (segment_reduce).

Math (reference):
    q = relu(query @ W1.T + b1) @ W2.T + b2          (B, d)
    s = relu(support @ W1.T + b1) @ W2.T + b2        (S, d)
    protos[c] = mean of s rows with label c (0 if empty)
    out[b, c] = -||q_b - protos_c||^2

Kernel strategy (8 NeuronCores, SPMD):
  - Shard both query (B) and support (S) over the 8 cores.
  - Inputs are host-packed so each tensor loads with ONE dma_start (the
    HWDGE sync queue issues serially at ~0.6us per call, and the
    runtime's collective entry barrier is queued behind them).
  - Support side per core: h = relu(x @ W1.T) (rows on partitions),
    one-hot(labels) built on-device (iota + is_equal).  Segment sums use
    the one-hot as the STATIONARY matmul operand: one accumulating matmul
    per 128-row chunk, out[100, 257] += onehot.T @ [h | ones]  (the ones
    column yields per-class counts in col 256).  W2 is applied once to
    the reduced (100, 256) sums (associativity).
  - AllGather the (100, 260) f16 partial [sums | counts] across cores,
    8-way summed locally with one DVE strided reduce.
  - b2 cancels exactly in ||q - p||^2; b1 applied via the free ACT bias
    on the query side and (only if b1 != 0, trace-time branch) an extra
    accumulating matmul on the support side.
  - Post-AG tail: scale sums by 2/max(cnt,1) (per-partition scalar since
    counts land as a [100,1] column), PE-transpose to [d, c], apply W2,
    then distances with the PROTOTYPES as stationary operands and the
    query streaming 512 columns at a time (6 matmuls total):
        G[c, b] = 2 p.q - pn - qn   accumulated per 512-query block
    The output is produced transposed ([C, BL] per core) and un-transposed
    on the host after the gather.
  - dtypes: support x / W1 / h / onehot in bf16; query chain + prototype
    path in float32r.  DRAM inputs on the f32r path are pre-rounded on
    the host (BIR requires f32r matmul operands produced as f32r).
"""

import numpy as np
import ml_dtypes
from contextlib import ExitStack

import concourse.bass as bass
import concourse.bacc as bacc
import concourse.mybir as mybir
import concourse.tile as tile
from concourse import bass_utils, masks

BF16 = mybir.dt.bfloat16
F32 = mybir.dt.float32
F32R = mybir.dt.float32r
I32 = mybir.dt.int32
F16 = mybir.dt.float16
F8 = mybir.dt.float8e4
AF = mybir.ActivationFunctionType
OP = mybir.AluOpType
AX = mybir.AxisListType

B, S, D, C = 8192, 65536, 256, 100
NCORES = 8
BL, SL = B // NCORES, S // NCORES          # 1024 query rows, 8192 support rows
NCH = SL // 128                            # 64 support chunks / core
CP = 260                                   # padded AG cols
CR = 104                                   # padded AG rows (104*260 % 32 == 0)


def f32r_round(x):
    """Round fp32 values to the fp32r grid (nearest, ~12-bit mantissa)."""
    x = np.ascontiguousarray(x, dtype=np.float32)
    try:
        from neuron_dtypes._impl import fp32r as _m
        u = x.reshape(-1).view(np.uint32)
        r = np.asarray(_m.cast_fp32_to_fp32r(len(u), u), dtype=np.uint32)
        b = np.asarray(_m.cast_fp32r_to_fp32(len(r), r), dtype=np.uint32)
        return b.view(np.float32).reshape(x.shape)
    except Exception:
        u = x.reshape(-1).view(np.uint32).astype(np.uint64)
        r = ((u + 0x800) & ~np.uint64(0xFFF)).astype(np.uint32)
        return r.view(np.float32).reshape(x.shape)


def pack_halves(a):
    """[256, N] -> [128, 2*N]: row p holds [k=0 half | k=1 half]."""
    n = a.shape[1]
    return np.ascontiguousarray(
        a.reshape(2, 128, n).transpose(1, 0, 2).reshape(128, 2 * n))


def build_program(b1_nonzero: bool):
    nc = bacc.Bacc("TRN2", target_bir_lowering=False, debug=False,
                   num_devices=NCORES)

    # packed inputs: one dma_start per tensor
    xt = nc.dram_tensor("xt_sup", [128, 2 * SL], F8,
                        kind="ExternalInput").ap()   # chunk-major [ci, k, j]
    lab = nc.dram_tensor("lab", [128, NCH], F32, kind="ExternalInput").ap()
    xqt = nc.dram_tensor("xqt", [128, 2 * BL], F32R,
                         kind="ExternalInput").ap()
    w1tb = nc.dram_tensor("w1t_bf", [128, 2 * D], F8,
                          kind="ExternalInput").ap()
    w1tf = nc.dram_tensor("w1t_f", [128, 2 * D], F32R,
                          kind="ExternalInput").ap()
    w2tf = nc.dram_tensor("w2t_f", [128, 2 * D], F32R,
                          kind="ExternalInput").ap()
    b1c = nc.dram_tensor("b1c", [128, 2], F32, kind="ExternalInput").ap()
    b1rb = nc.dram_tensor("b1r_bf", [1, D], BF16, kind="ExternalInput").ap()
    out = nc.dram_tensor("out", [C, BL], F16, kind="ExternalOutput").ap()

    with tile.TileContext(nc) as tc, ExitStack() as ctx:
        sb = ctx.enter_context(tc.tile_pool(name="sb", bufs=1))
        dram = ctx.enter_context(tc.tile_pool(name="dram", bufs=1, space="DRAM"))
        hpool = ctx.enter_context(tc.tile_pool(name="hpool", bufs=3))
        ohpool = ctx.enter_context(tc.tile_pool(name="ohpool", bufs=3))
        opool = ctx.enter_context(tc.tile_pool(name="opool", bufs=2))

        # ---- weights / support stream (9 serial dma_start issuances total)
        w1tb_sb = sb.tile([128, 2 * D], F8, name="w1tb_sb")
        nc.sync.dma_start(w1tb_sb[:], w1tb[:])
        lab_sb = sb.tile([128, NCH], F32, name="lab_sb")
        nc.sync.dma_start(lab_sb[:], lab[:])
        xt_sb = sb.tile([128, 2 * SL], F8, name="xt_sb")
        nc.sync.dma_start(xt_sb[:, 0:2048], xt[:, 0:2048])
        nc.sync.dma_start(xt_sb[:, 2048:8192], xt[:, 2048:8192])
        nc.sync.dma_start(xt_sb[:, 8192:2 * SL], xt[:, 8192:2 * SL])
        xqt_sb = sb.tile([128, 2 * BL], F32R, name="xqt_sb")
        nc.sync.dma_start(xqt_sb[:], xqt[:])
        w1tf_sb = sb.tile([128, 2 * D], F32R, name="w1tf_sb")
        nc.sync.dma_start(w1tf_sb[:], w1tf[:])
        w2tf_sb = sb.tile([128, 2 * D], F32R, name="w2tf_sb")
        nc.sync.dma_start(w2tf_sb[:], w2tf[:])
        b1c_sb = sb.tile([128, 2], F32, name="b1c_sb")
        nc.sync.dma_start(b1c_sb[:], b1c[:])

        iota_i = sb.tile([128, C], I32, name="iota_i")
        nc.gpsimd.iota(iota_i[:], pattern=[[1, C]], base=0, channel_multiplier=0)
        iota_f = sb.tile([128, C], F32, name="iota_f")
        nc.vector.tensor_copy(iota_f[:], iota_i[:])
        if b1_nonzero:
            ones_row_bf = sb.tile([1, 128], BF16, name="ones_row_bf")
            nc.vector.memset(ones_row_bf[:], 1.0)
            b1rb_sb = sb.tile([1, D], BF16, name="b1rb_sb")
            nc.sync.dma_start(b1rb_sb[:], b1rb[:])

        # ---- PE warm-up: dummy matmuls on a memset tile so the HAM clock
        # gate flips to 2.4 GHz before the real work arrives
        scratch_bf = sb.tile([128, 512], BF16, name="scratch_bf")
        nc.vector.memset(scratch_bf[:], 0.5)
        with tc.tile_pool(name="pwarm", bufs=1, space="PSUM") as pwarm:
            warm_ps = pwarm.tile([128, 512], F32, name="warm_ps")
            for _ in range(5):
                nc.tensor.matmul(warm_ps[:], scratch_bf[:, 0:128],
                                 scratch_bf[:], start=True, stop=True,
                                 skip_group_check=True)

        def x_slice(k, ci):
            base = ci * 256 + k * 128
            return xt_sb[:, base:base + 128]

        # ---- distance-stage f32r constants (DVE copy from f32 memsets,
        # which the BIR verifier accepts as a valid f32r producer)
        onesq_f = sb.tile([1, BL], F32, name="onesq_f")
        nc.vector.memset(onesq_f[:], 1.0)
        ones_q = sb.tile([1, BL], F32R, name="ones_q")
        nc.vector.tensor_copy(ones_q[:], onesq_f[:])
        e1f = sb.tile([1, 2], F32, name="e1f")
        nc.vector.memset(e1f[:, 0:1], 0.0)
        nc.vector.memset(e1f[:, 1:2], 1.0)
        e1_r = sb.tile([1, 2], F32R, name="e1_r")
        nc.vector.tensor_copy(e1_r[:], e1f[:])
        cm025_2 = sb.tile([128, 2], F32, name="cm025_2")
        nc.vector.memset(cm025_2[:, 0:1], 0.0)
        nc.vector.memset(cm025_2[:, 1:2], -0.25)
        em1 = sb.tile([1, 2], F32, name="em1")
        nc.vector.memset(em1[:, 0:1], -1.0)
        nc.vector.memset(em1[:, 1:2], 0.0)
        ones_c = sb.tile([1, C], F32, name="ones_c")
        nc.vector.memset(ones_c[:], 1.0)
        oc2_f = sb.tile([128, 2], F32, name="oc2_f")
        nc.vector.memset(oc2_f[:, 0:1], 1.0)
        nc.vector.memset(oc2_f[:, 1:2], 0.0)
        onescol2 = sb.tile([128, 2], F32R, name="onescol2")
        nc.vector.tensor_copy(onescol2[:], oc2_f[:])
        ident = sb.tile([128, 128], F32, name="ident")
        masks.make_identity(nc, ident[:])

        # partial [sums | cnt | pad] staged for the AllGather, on an fp8
        # wire: values scaled by 1/64 into e4m3 range (sums are 16x from
        # the W1 host scaling; class sums stay well under 240*64)
        part_sb = sb.tile([CR, CP], F8, name="part_sb")
        nc.vector.memset(part_sb[:], 0.0)

        # ================= support phase =================
        # seg_ps[100, 257] accumulates onehot.T @ [h | ones] over all 64
        # chunks: cols 0:256 are per-class h sums, col 256 the counts.
        with tc.tile_pool(name="ph", bufs=3, space="PSUM") as ph, \
             tc.tile_pool(name="pacc", bufs=1, space="PSUM") as pacc:
            seg_ps = pacc.tile([C, D + 1], F32, name="seg_ps")
            for ci in range(NCH):
                h_ps = ph.tile([128, D], F32, name="h_ps")
                nc.tensor.matmul(h_ps[:], x_slice(0, ci), w1tb_sb[:, 0:D],
                                 start=True, stop=False)
                nc.tensor.matmul(h_ps[:], x_slice(1, ci), w1tb_sb[:, D:2 * D],
                                 start=False, stop=not b1_nonzero)
                if b1_nonzero:
                    nc.tensor.matmul(h_ps[:], ones_row_bf[:], b1rb_sb[:],
                                     start=False, stop=True)
                h_sb = hpool.tile([128, D + 1], BF16, name="h_sb")
                nc.scalar.activation(h_sb[:, 0:D], h_ps[:], AF.Relu)
                nc.vector.memset(h_sb[:, D:D + 1], 1.0)
                oh = ohpool.tile([128, C], BF16, name="oh")
                nc.vector.tensor_scalar(oh[:], iota_f[:], lab_sb[:, ci:ci + 1],
                                        None, OP.is_equal)
                nc.tensor.matmul(seg_ps[:], oh[:], h_sb[:],
                                 start=ci == 0, stop=ci == NCH - 1)
            nc.vector.tensor_scalar(part_sb[0:C, 0:D + 1], seg_ps[:],
                                    1.0 / 64.0, None, OP.mult)

        # ========== AllGather partial [sums | cnt], reduce locally ======
        arin = dram.tile([CR, CP], F8, name="arin")
        arout = dram.tile([NCORES * CR, CP], F8, name="arout")
        nc.sync.dma_start(arin[:], part_sb[:])
        nc.gpsimd.collective_compute(
            "AllGather", OP.bypass,
            replica_groups=[list(range(NCORES))],
            ins=[arin.opt()], outs=[arout.opt()])

        # ================= query projector (overlaps the AG window) ======
        hq_sb = [sb.tile([128, BL], F32R, name=f"hq{k}") for k in range(2)]
        qt_sb = [sb.tile([128, BL], F32R, name=f"qt{k}") for k in range(2)]
        qsq_sb = [sb.tile([128, BL], F32R, name=f"qsq{k}") for k in range(2)]
        # qn2: row 0 = ||q||^2, row 1 = ones -> distance rhs K-chunk of 2
        qn2 = sb.tile([2, BL], F32R, name="qn2")
        with tc.tile_pool(name="pq", bufs=2, space="PSUM") as pq:
            for qb in range(2):
                qsl = slice(qb * 512, (qb + 1) * 512)
                for mc in range(2):
                    msl = slice(mc * 128, (mc + 1) * 128)
                    hq_ps = pq.tile([128, 512], F32, name="hq_ps")
                    for kc in range(2):
                        nc.tensor.matmul(
                            hq_ps[:],
                            w1tf_sb[:, kc * D + mc * 128:kc * D + mc * 128 + 128],
                            xqt_sb[:, kc * BL + qb * 512:kc * BL + qb * 512 + 512],
                            start=kc == 0, stop=kc == 1)
                    nc.scalar.activation(hq_sb[mc][:, qsl], hq_ps[:], AF.Relu,
                                         bias=b1c_sb[:, mc:mc + 1])
                for mc in range(2):
                    qt_ps = pq.tile([128, 512], F32, name="qt_ps")
                    for kc in range(2):
                        nc.tensor.matmul(
                            qt_ps[:],
                            w2tf_sb[:, kc * D + mc * 128:kc * D + mc * 128 + 128],
                            hq_sb[kc][:, qsl],
                            start=kc == 0, stop=kc == 1)
                    nc.scalar.copy(qt_sb[mc][:, qsl], qt_ps[:])
                    nc.scalar.square(qsq_sb[mc][:, qsl], qt_ps[:])
                qn_ps = pq.tile([2, 512], F32, name="qn_ps")
                for kc in range(2):
                    nc.tensor.matmul(qn_ps[:], onescol2[:],
                                     qsq_sb[kc][:, qsl],
                                     start=kc == 0, stop=False)
                nc.tensor.matmul(qn_ps[:], e1_r[:], ones_q[:, qsl],
                                 start=False, stop=True)
                nc.scalar.copy(qn2[:, qsl], qn_ps[:])

        # ================= combine partials (needs the AG result) =========
        # gather split into two column-half DMAs on separate HWDGE queues:
        # the counts-carrying half lands first so the count chain, diag and
        # the kc=1 transpose/W2 step run while cols 0:128 are still in
        # flight.
        ar_view = arout.rearrange("(r z) j -> r z j", z=CR)
        gk = sb.tile([C, NCORES, CP], F8, name="gk")
        nc.sync.dma_start(
            gk[:, :, 128:CP],
            ar_view[:, 0:C, 128:CP].rearrange("r p j -> p r j"))
        nc.scalar.dma_start(
            gk[:, :, 0:128],
            ar_view[:, 0:C, 0:128].rearrange("r p j -> p r j"))
        # count column reduced first in a tiny parallel chain so the
        # reciprocal is ready before the big add tree finishes
        c1 = sb.tile([C, 4, 1], F32, name="c1")
        nc.vector.tensor_tensor(c1[:], gk[:, 0:4, D:D + 1],
                                gk[:, 4:8, D:D + 1], OP.add)
        c2t = sb.tile([C, 2, 1], F32, name="c2t")
        nc.vector.tensor_tensor(c2t[:], c1[:, 0:2, :], c1[:, 2:4, :], OP.add)
        ccol = sb.tile([C, 1], F32, name="ccol")
        nc.vector.tensor_tensor(ccol[:], c2t[:, 0:1, :].squeeze(1),
                                c2t[:, 1:2, :].squeeze(1), OP.add)
        a1 = sb.tile([C, 4, CP], F16, name="a1")
        a2 = sb.tile([C, 2, CP], F16, name="a2")
        hsr = sb.tile([C, CP], F32, name="hsr")
        for lo, hi in ((128, CP), (0, 128)):
            nc.vector.tensor_tensor(a1[:, :, lo:hi], gk[:, 0:4, lo:hi],
                                    gk[:, 4:8, lo:hi], OP.add)
            nc.vector.tensor_tensor(a2[:, :, lo:hi], a1[:, 0:2, lo:hi],
                                    a1[:, 2:4, lo:hi], OP.add)
            nc.vector.tensor_tensor(hsr[:, lo:hi],
                                    a2[:, 0:1, lo:hi].squeeze(1),
                                    a2[:, 1:2, lo:hi].squeeze(1), OP.add)

        # ---- prototypes: scale by 2/max(cnt,1), transpose, apply W2
        cmax = sb.tile([C, 1], F32, name="cmax")
        inv2 = sb.tile([C, 1], F32, name="inv2")
        hmr = [sb.tile([128, C], F32R, name=f"hmr{k}") for k in range(2)]
        p2t_r = [sb.tile([128, C], F32R, name=f"p2tr{k}") for k in range(2)]
        p2sq_sb = [sb.tile([128, C], F32, name=f"p2sq{k}") for k in range(2)]
        # pnm: row 0 = -ones, row 1 = -||p||^2 -> distance lhsT K-chunk of 2
        pnm = sb.tile([2, C], F32R, name="pnm")
        # distances are interleaved with the pnm preparation: the 4 big
        # query-x-prototype matmuls only need p2t_r, so the PE runs them
        # while ACT squares the prototypes and the small pnm matmuls
        # follow; each distance PSUM group closes with its pnm term.
        with tc.tile_pool(name="pp", bufs=1, space="PSUM") as pp:
            # cmax = max(cnt, 1) * 8 (fp8 wire + W1x16 scales folded in);
            # the per-class scale rides the transpose matmul as diag(inv2)
            nc.vector.tensor_scalar(cmax[:], ccol[:], 8.0, 0.125,
                                    OP.mult, OP.max)
            nc.vector.reciprocal(inv2[:], cmax[:])
            diag_sb = sb.tile([C, C], F32, name="diag_sb")
            nc.vector.tensor_scalar(diag_sb[:], ident[:C, :C], inv2[:],
                                    None, OP.mult)
            for kc in (1, 0):
                ht_ps = pp.tile([128, C], F32, name=f"ht{kc}")
                # plain fp32 matmul: hsr_slice.T @ diag(inv2) transposes AND
                # applies the per-class scale in one PE op.  kc=1 first: its
                # hsr columns arrive with the first gather half.
                nc.tensor.matmul(ht_ps[:], hsr[:, kc * 128:(kc + 1) * 128],
                                 diag_sb[:], start=True, stop=True)
                nc.vector.tensor_copy(hmr[kc][:], ht_ps[:])
            for mc in range(2):
                p2_ps = pp.tile([128, C], F32, name=f"p2_{mc}")
                for idx, kc in enumerate((1, 0)):
                    nc.tensor.matmul(
                        p2_ps[:],
                        w2tf_sb[:, kc * D + mc * 128:kc * D + mc * 128 + 128],
                        hmr[kc][:],
                        start=idx == 0, stop=idx == 1)
                nc.vector.tensor_copy(p2t_r[mc][:], p2_ps[:])
                nc.scalar.square(p2sq_sb[mc][:], p2_ps[:])
            g_ps = [pp.tile([C, 512], F32, name=f"g{h}") for h in range(2)]
            for half in range(2):
                qsl = slice(half * 512, (half + 1) * 512)
                nc.tensor.matmul(g_ps[half][:], p2t_r[0][:], qt_sb[0][:, qsl],
                                 start=True, stop=False)
                nc.tensor.matmul(g_ps[half][:], p2t_r[1][:], qt_sb[1][:, qsl],
                                 start=False, stop=False)
            mpn_ps = pp.tile([2, C], F32, name="mpn_ps")
            for mc in range(2):
                nc.tensor.matmul(mpn_ps[:], cm025_2[:], p2sq_sb[mc][:],
                                 start=mc == 0, stop=False)
            nc.tensor.matmul(mpn_ps[:], em1[:], ones_c[:],
                             start=False, stop=True)
            nc.vector.tensor_copy(pnm[:], mpn_ps[:])
            for half in range(2):
                qsl = slice(half * 512, (half + 1) * 512)
                nc.tensor.matmul(g_ps[half][:], pnm[:], qn2[:, qsl],
                                 start=False, stop=True)
                o_sb = opool.tile([C, 512], F16, name="o_sb")
                nc.vector.tensor_copy(o_sb[:], g_ps[half][:])
                nc.sync.dma_start(out[:, qsl], o_sb[:])

    nc.compile()
    return nc


def make_in_maps(query, support_feats, support_labels, W1, b1, W2, b2):
    q = np.ascontiguousarray(np.asarray(query, dtype=np.float32))
    x = np.asarray(support_feats, dtype=np.float32)
    labels = np.asarray(support_labels).astype(np.int64)
    W1 = np.asarray(W1, dtype=np.float32)
    b1 = np.asarray(b1, dtype=np.float32)
    W2 = np.asarray(W2, dtype=np.float32)

    w1t_f8 = pack_halves(np.ascontiguousarray(W1.T * 16.0)).astype(
        ml_dtypes.float8_e4m3)
    w1t_f = f32r_round(pack_halves(np.ascontiguousarray(W1.T)))
    w2t_f = f32r_round(pack_halves(np.ascontiguousarray(W2.T)))
    b1col = pack_halves(np.ascontiguousarray(b1.reshape(D, 1)))
    b1row_bf = np.ascontiguousarray(16.0 * b1.reshape(1, D)).astype(
        ml_dtypes.bfloat16)

    in_maps = []
    for c in range(NCORES):
        xs = x[c * SL:(c + 1) * SL]
        ls = labels[c * SL:(c + 1) * SL]
        xqs = q[c * BL:(c + 1) * BL]
        # xt chunk-major: [p, ci*256 + k*128 + j] = xs[ci*128 + j, k*128 + p]
        xt_pack = np.ascontiguousarray(
            xs.T.reshape(2, 128, NCH, 128).transpose(1, 2, 0, 3)
            .reshape(128, 2 * SL)).astype(ml_dtypes.float8_e4m3)
        in_maps.append({
            "xt_sup": xt_pack,
            "lab": np.ascontiguousarray(
                ls.reshape(NCH, 128).T.astype(np.float32)),
            "xqt": f32r_round(pack_halves(np.ascontiguousarray(xqs.T))),
            "w1t_bf": w1t_f8,
            "w1t_f": w1t_f,
            "w2t_f": w2t_f,
            "b1c": b1col,
            "b1r_bf": b1row_bf,
        })
    return in_maps


_cached = {}


def _get_program(b1_nonzero: bool):
    key = bool(b1_nonzero)
    if key not in _cached:
        _cached[key] = build_program(key)
    return _cached[key]


def kernel(query, support_feats, support_labels, W1, b1, W2, b2,
           **run_kwargs):
    b1_nonzero = bool(np.any(np.asarray(b1)))
    nc = _get_program(b1_nonzero)
    in_maps = make_in_maps(query, support_feats, support_labels,
                           W1, b1, W2, b2)
    res = bass_utils.run_bass_kernel_spmd(
        nc, in_maps, core_ids=list(range(NCORES)), **run_kwargs)
    out = np.concatenate(
        [np.ascontiguousarray(res.results[c]["out"].T) for c in range(NCORES)],
        axis=0)
    return out.astype(np.float32, copy=False)


if __name__ == "__main__":
    import sys
    sys.path.insert(0, "/root/problem")
    from reference import setup_inputs
    inputs = {k: np.asarray(v) for k, v in setup_inputs().items()}
    o = kernel(**inputs)
    print("out", o.shape, o.dtype, o[:2, :4])
